# revision 12
# baseline (speedup 1.0000x reference)
"""Trainium2 Bass kernel for a multi-head-attention block (B,C,N,D = 8,4,1024,96;
H=3 heads, dk=dv=32; softmax over the QUERY axis; residual + LayerNorm).

Sharding: pure data-parallel over batch B across 8 NeuronCores (one batch
element per core, C=4 channel-slices each, no collectives).

Schedule notes:
  - ACT (exp) is the pacing engine: 96 exps x ~1.1us + 96 accumulator
    reads (~0.3us, unavoidable: one accumulator register and the q-axis
    softmax denominators can only come from the ACT accumulator).
    Everything else is scheduled to hide under it.
  - per (c,i) slot emission order: scores+exp first (three heads' matmuls
    adjacent at row-groups 0/32/64 -> concurrent in the PE array), then
    ctx(i-1) (col-groups, concurrent), then phase-1/tail fillers;
    w_psum ring allocations padded to multiples of 3 so scores tiles
    always wait on the matching head's exp
  - channel-0 prologue: DMAs split across the sync/scalar/gpsimd queues
    (scalar's queue is free before the first exp), PE spin matmuls keep
    the p-state ramped through the DMA window, and only what scores(0,0)
    needs is built before the first exp
  - no DMA triggers on the Scalar engine after exps start (each costs the
    exp engine ~670ns); weights arrive host-pre-transposed in wall (no PE
    weight transposes)
  - LN tail: residual via scalar_tensor_tensor, bn_stats/bn_aggr for
    mean/var, rsqrt via bit-hack + 2 Newton iterations (all DVE);
    channel-3 tail is pipelined per q-half with split store DMAs
"""

from contextlib import ExitStack

import numpy as np

import concourse.bass as bass
import concourse.tile as tile
from concourse import bacc, mybir
from concourse.bass_utils import run_bass_kernel_spmd

F32 = mybir.dt.float32
BF16 = mybir.dt.bfloat16
F32R = mybir.dt.float32r
I32 = mybir.dt.int32
I16 = mybir.dt.int16
A = mybir.AluOpType

B, C, N, D = 8, 4, 1024, 96
H, DK, DV = 3, 32, 32
P = 128               # partition size / token chunk
NCHUNK = N // P       # 8
QT = 512              # matmul free-dim limit into one PSUM bank (f32)
SCALE = 1.0 / np.sqrt(DK)
EPS = 1e-5

# Schraudolph exp on the DVE: bits(bf16 e) = round(x*SCALE*A + B) as int16.
# Per-tile constant factors cancel exactly in the q-axis softmax (each
# k-row's numerator and denominator live in one tile), so the ~3-6%
# pointwise exp error averages to ~1e-3 relative in the final output.
SCH_A = float((2.0 ** 7) / np.log(2.0) * SCALE)
SCH_B = float(127 * 2 ** 7)


def _dve_tile(c, i):
    """Chunks whose head-2 exp runs on the DVE instead of ACT (the
    engine-balance knob).  DVE is busiest during channel-0 prep, so skip
    the earliest chunks."""
    return c >= 1 or i >= 6

_CACHE = {}


def _emit(nc, tc, ctx, apply_affine):
    xq_d = nc.dram_tensor("xq", [C, N, D], F32R, kind="ExternalInput").ap()
    xk_d = nc.dram_tensor("xk", [C, N, D], F32R, kind="ExternalInput").ap()
    xv_d = nc.dram_tensor("xv", [C, N, D], F32R, kind="ExternalInput").ap()
    # wall = host-packed [128, 128 + 4*96 + 2*1]: identity | wq|wk|wv|wfc
    # (each [96,96] natural, zero-padded to 128 rows) | gamma | beta columns
    wall_d = nc.dram_tensor("wall", [P, P + 4 * D + 2], F32R,
                            kind="ExternalInput").ap()
    out_d = nc.dram_tensor("out", [C, N, D], F32, kind="ExternalOutput").ap()

    const = ctx.enter_context(tc.tile_pool(name="const", bufs=1))
    pc = ctx.enter_context(tc.tile_pool(name="perc", bufs=2))
    w_psum = ctx.enter_context(tc.tile_pool(name="w_psum", bufs=3, space="PSUM"))
    ctx_psum = ctx.enter_context(tc.tile_pool(name="ctx_psum", bufs=1, space="PSUM"))

    # ---- w_psum ring discipline: pad allocations to multiples of 3 so the
    # 3 scores tiles of chunk i+1 land exactly on the banks freed by the
    # 3 exps of chunk i (same head -> earliest possible reuse).
    wct = {"n": 0, "pad": 0}

    def wtile(shape, name):
        wct["n"] += 1
        return w_psum.tile(shape, F32, name=name, tag="w")

    def wpad():
        while wct["n"] % 3:
            wct["n"] += 1
            wct["pad"] += 1
            w_psum.tile([P, 8], F32, name=f"pad{wct['pad']}", tag="w")

    # ---------------- prologue: DMAs + PE warm-up spins ----------------
    # dummy tile for spin matmuls (keeps the PE busy through the initial
    # DMA window so the p-state/HAM ramps before the real transposes)
    dummy = const.tile([P, QT], BF16)
    nc.vector.memset(dummy, 0)

    # split the wall DMA so the identity (gates the first transposes)
    # arrives before the weight columns
    wall = const.tile([P, P + 4 * D + 2], F32R)
    nc.sync.dma_start(out=wall[:, 0:P], in_=wall_d[:, 0:P])
    nc.sync.dma_start(out=wall[:, P:], in_=wall_d[:, P:])
    ident = wall[:, 0:P]

    # channel-0 loads, split for earliest availability of the q/k paths
    xq0 = pc.tile([P, NCHUNK, D], F32R, name="xq_nat0", tag="xq_nat", bufs=3)
    xk0 = pc.tile([P, NCHUNK, D], F32R, name="xk_nat0", tag="xk_nat", bufs=1)
    xv0 = pc.tile([P, NCHUNK, D], F32R, name="xv_nat0", tag="xv_nat", bufs=1)
    xq0_r = xq_d[0].rearrange("(i p) d -> p i d", p=P)
    xk0_r = xk_d[0].rearrange("(i p) d -> p i d", p=P)
    # ACT is idle through the prologue, so its DMA queue is free here
    # (never used for DMAs once exps start); one DMA per queue to avoid
    # per-queue DGE serialization (~1.5us per DMA)
    nc.scalar.dma_start(out=xq0[:, 0:4, :], in_=xq0_r[:, 0:4, :])
    nc.gpsimd.dma_start(out=xq0[:, 4:8, :], in_=xq0_r[:, 4:8, :])
    nc.gpsimd.dma_start(out=xk0[:, 0:4, :], in_=xk0_r[:, 0:4, :])
    nc.gpsimd.dma_start(out=xk0[:, 4:8, :], in_=xk0_r[:, 4:8, :])
    nc.gpsimd.dma_start(out=xv0, in_=xv_d[0].rearrange("(i p) d -> p i d", p=P))

    spin = wtile([P, QT], "spin")
    for _ in range(4):
        nc.tensor.matmul(spin[0:64, :], lhsT=dummy[:, 0:64], rhs=dummy,
                         start=True, stop=True, skip_group_check=True)

    # weights arrive host-pre-transposed in wall (W.T packed): zero PE work
    wts = {
        nm: wall[0:D, P + k * D:P + (k + 1) * D]
        for k, nm in enumerate(("wq", "wk", "wv", "wfc"))
    }
    wpad()

    gam_tile = bet_tile = None
    if apply_affine:
        gam_tile = const.tile([P, D], F32)
        bet_tile = const.tile([P, D], F32)
        for t, col in ((gam_tile, P + 4 * D), (bet_tile, P + 4 * D + 1)):
            col_ap = wall_d[0:D, col:col + 1]
            bcast = bass.AP(tensor=col_ap.tensor, offset=col_ap.offset,
                            ap=[[0, P], col_ap.ap[0]])
            nc.gpsimd.dma_start(out=t, in_=bcast)

    st = {0: dict(xq_nat=xq0, xk_nat=xk0, xv_nat=xv0, xTs={})}
    ts = {}

    def alloc_chunk_state(c):
        s = st[c]
        s["ssum"] = pc.tile([P, H * NCHUNK], F32, name=f"ssum{c}", tag="ssum")
        s["e_all"] = pc.tile([P, H * NCHUNK, N], BF16, name=f"e{c}", tag="e")
        s["vsc_all"] = pc.tile([P, H * NCHUNK, DV], BF16, name=f"vsc{c}",
                               tag="vsc")

    def loads(c):
        """DMA loads for channel c (c >= 1): no triggers on the Scalar eng."""
        xq = pc.tile([P, NCHUNK, D], F32R, name=f"xq_nat{c}", tag="xq_nat",
                     bufs=3)
        xk = pc.tile([P, NCHUNK, D], F32R, name=f"xk_nat{c}", tag="xk_nat",
                     bufs=1)
        xv = pc.tile([P, NCHUNK, D], F32R, name=f"xv_nat{c}", tag="xv_nat",
                     bufs=1)
        nc.sync.dma_start(out=xq, in_=xq_d[c].rearrange("(i p) d -> p i d", p=P))
        nc.gpsimd.dma_start(out=xk, in_=xk_d[c].rearrange("(i p) d -> p i d", p=P))
        nc.gpsimd.dma_start(out=xv, in_=xv_d[c].rearrange("(i p) d -> p i d", p=P))
        st[c] = dict(xq_nat=xq, xk_nat=xk, xv_nat=xv, xTs={})

    def tr_g(c, nm, g):
        """transpose 4 chunks (group g) of one input -> X.T[:, 512g:512(g+1)]"""
        s = st[c]
        src = s[f"x{nm}_nat"]
        if nm not in s["xTs"]:
            s["xTs"][nm] = pc.tile([D, N], F32R, name=f"x{nm}T{c}",
                                   tag=f"x{nm}T", bufs=1)
        xT = s["xTs"][nm]
        tp_ps = wtile([D, 4 * P], f"tp{nm}{c}{g}")
        for j in range(4):
            i = 4 * g + j
            nc.tensor.matmul(tp_ps[:, j * P:(j + 1) * P],
                             lhsT=src[:, i, :], rhs=ident,
                             start=True, stop=True)
        nc.vector.tensor_copy(out=xT[:, g * 4 * P:(g + 1) * 4 * P], in_=tp_ps)

    def proj_qk(c, which, g):
        """Q or K projection into [e, tok] bf16 layout, qtile g"""
        s = st[c]
        nm, w_t = (("qdT", wts["wq"]) if which == "q" else ("kdT", wts["wk"]))
        if nm not in s:
            s[nm] = pc.tile([D, N], BF16, name=f"{nm}{c}", tag=nm)
        dst, xT = s[nm], s["xTs"][which]
        pr_ps = wtile([D, QT], f"pr{c}{which}{g}")
        nc.tensor.matmul(pr_ps, lhsT=w_t, rhs=xT[:, g * QT:(g + 1) * QT],
                         start=True, stop=True)
        nc.vector.tensor_copy(out=dst[:, g * QT:(g + 1) * QT], in_=pr_ps)

    def v_g(c, g):
        """V projection (natural layout), chunks 4g..4g+3 (bf16 W_V moving)"""
        s = st[c]
        if "v_nat" not in s:
            s["v_nat"] = pc.tile([P, NCHUNK, D], F32, name=f"v_nat{c}",
                                 tag="v_nat")
        v_nat = s["v_nat"]
        v_ps = wtile([P, 4 * D], f"vps{c}{g}")
        for j in range(4):
            i = 4 * g + j
            nc.tensor.matmul(v_ps[:, j * D:(j + 1) * D],
                             lhsT=s["xTs"]["v"][:, i * P:(i + 1) * P],
                             rhs=wts["wv"], start=True, stop=True)
        nc.vector.tensor_copy(
            out=v_nat[:, 4 * g:4 * (g + 1), :].rearrange("p i d -> p (i d)"),
            in_=v_ps)

    pending_reduce = []

    def exp_tile(c, i, h):
        """exp for one (chunk, head) score tile.  ACT: spline exp + fused
        accumulator (denominator).  DVE (head 2 of offloaded chunks):
        Schraudolph bit-hack tensor_scalar; the tensor_reduce denominator
        is deferred to the slot end (it has a full slot of slack before
        the gpsimd norm_recip needs it) so the slot's cast fillers -- which
        gate the PSUM scores ring -- run first on the in-order DVE."""
        s = st[c]
        j = i * H + h
        if h == 2 and _dve_tile(c, i):
            nc.vector.tensor_scalar(
                out=s["e_all"][:, j, :].bitcast(I16), in0=s["s_regs"][h],
                scalar1=SCH_A, scalar2=SCH_B, op0=A.mult, op1=A.add)
            pending_reduce.append((c, j))
        else:
            nc.scalar.activation(
                out=s["e_all"][:, j, :], in_=s["s_regs"][h],
                func=mybir.ActivationFunctionType.Exp,
                scale=SCALE, accum_out=s["ssum"][:, j:j + 1])

    def flush_reduce():
        while pending_reduce:
            c_, j_ = pending_reduce.pop(0)
            s_ = st[c_]
            nc.vector.tensor_reduce(
                out=s_["ssum"][:, j_:j_ + 1], in_=s_["e_all"][:, j_, :],
                axis=mybir.AxisListType.X, op=A.add)

    def scores_exp(c, i):
        """S_T + exp for chunk i.  The three heads' matmuls are adjacent at
        row-groups 0/32/64 so they run concurrently in the PE array."""
        s = st[c]
        s_regs = [wtile([P, N], f"s{c}_{i}_{h}") for h in range(H)]
        s["s_regs"] = s_regs
        for g in range(2):
            for h in range(H):
                hs = slice(DK * h, DK * (h + 1))
                nc.tensor.matmul(
                    s_regs[h][:, g * QT:(g + 1) * QT],
                    lhsT=s["kdT"][hs, i * P:(i + 1) * P],
                    rhs=s["qdT"][hs, g * QT:(g + 1) * QT],
                    start=True, stop=True)
        for h in range(H):
            exp_tile(c, i, h)

    def vsc_mm(c, i):
        """fold 1/denom into V rows for chunk i: gpsimd normalize_recip
        computes vsc = v/denom and overwrites ssum with 1/denom in place
        (the reciprocal is a free by-product; nothing reads it)."""
        s = st[c]
        for h in range(H):
            hs = slice(DV * h, DV * (h + 1))
            j = i * H + h
            nc.gpsimd.normalize_recip(
                out_ap=s["vsc_all"][:, j, :], in_ap=s["v_nat"][:, i, hs],
                denom_ap=s["ssum"][:, j:j + 1])

    def ctx_mm(c, i):
        """context accumulation for chunk i: bf16, three heads at
        col-groups 0/32/64, emitted adjacently -> concurrent."""
        s = st[c]
        for g in range(2):
            for h in range(H):
                hs = slice(DV * h, DV * (h + 1))
                j = i * H + h
                nc.tensor.matmul(
                    s["ctx_ps"][hs, g * QT:(g + 1) * QT],
                    lhsT=s["vsc_all"][:, j, :],
                    rhs=s["e_all"][:, j, g * QT:(g + 1) * QT],
                    start=(i == 0), stop=(i == NCHUNK - 1),
                    skip_group_check=True)

    def tail_a(c, half=None):
        """ctx copy-out (frees the ctx PSUM banks); half in {0,1,None}"""
        s = st[c]
        if "ctxT" not in s:
            s["ctxT"] = pc.tile([D, N], F32R, name=f"ctxT{c}", tag="ctxT")
        halves = (0, 1) if half is None else (half,)
        for g in halves:
            nc.vector.tensor_copy(out=s["ctxT"][:, g * QT:(g + 1) * QT],
                                  in_=s["ctx_ps"][:, g * QT:(g + 1) * QT])
        if half in (1, None):
            del s["ctx_ps"]

    def fc_group(c, g):
        """fc matmuls + residual add + one bn_stats pass for chunks 4g..4g+3"""
        s = st[c]
        if c not in ts:
            ts[c] = dict(
                t_all=pc.tile([P, NCHUNK, D], F32, name=f"tall{c}", tag="tall"),
                bst=pc.tile([P, NCHUNK, 6], F32, name=f"bst{c}", tag="bst"),
                mv=pc.tile([P, NCHUNK, 2], F32, name=f"mv{c}", tag="mv"),
            )
        t = ts[c]
        fc_ps = wtile([P, 4 * D], f"fc{c}{g}")
        for j in range(4):
            i = 4 * g + j
            nc.tensor.matmul(fc_ps[:, j * D:(j + 1) * D],
                             lhsT=s["ctxT"][:, i * P:(i + 1) * P],
                             rhs=wts["wfc"], start=True, stop=True)
        for j in range(4):
            i = 4 * g + j
            nc.vector.scalar_tensor_tensor(
                out=t["t_all"][:, i, :], in0=fc_ps[:, j * D:(j + 1) * D],
                scalar=1.0, in1=s["xq_nat"][:, i, :].bitcast(F32),
                op0=A.mult, op1=A.add)
            nc.vector.bn_stats(out=t["bst"][:, i, :], in_=t["t_all"][:, i, :])

    def ln_stats(c, lo=0, hi=NCHUNK):
        """per-chunk bn_aggr -> (mean, var); rstd via bit-hack + 2 Newton
        iterations (DVE)"""
        t = ts[c]
        if "var" not in t:
            t["var"] = pc.tile([P, NCHUNK], F32, name=f"var{c}", tag="var")
            t["y"] = pc.tile([P, NCHUNK], F32, name=f"y{c}", tag="y")
            t["t1"] = pc.tile([P, NCHUNK], F32, name=f"t1n{c}", tag="t1n")
        sl = slice(lo, hi)
        var, y, t1 = t["var"][:, sl], t["y"][:, sl], t["t1"][:, sl]
        for i in range(lo, hi):
            nc.vector.bn_aggr(out=t["mv"][:, i, :], in_=t["bst"][:, i, :])
        nc.vector.tensor_scalar_add(
            out=var, in0=t["mv"][:, sl, 1:2].rearrange("p i o -> p (i o)"),
            scalar1=EPS)
        nc.vector.tensor_scalar(
            out=y.bitcast(I32), in0=var.bitcast(I32), scalar1=1,
            scalar2=None, op0=A.logical_shift_right)
        nc.vector.tensor_scalar(
            out=y.bitcast(I32), in0=y.bitcast(I32), scalar1=-1,
            scalar2=None, op0=A.bitwise_xor)
        nc.vector.tensor_scalar(
            out=y.bitcast(I32), in0=y.bitcast(I32), scalar1=0x5F3759E0,
            scalar2=None, op0=A.add)
        for _ in range(2):
            nc.vector.tensor_mul(out=t1, in0=y, in1=y)
            nc.vector.tensor_mul(out=t1, in0=t1, in1=var)
            nc.vector.tensor_scalar(out=t1, in0=t1, scalar1=-0.5, scalar2=1.5,
                                    op0=A.mult, op1=A.add)
            nc.vector.tensor_mul(out=y, in0=y, in1=t1)

    def ln_out(c, lo, hi, on_act=False):
        """(t - mean) * rstd, chunks lo..hi-1.  on_act runs it on the
        Scalar engine as t*rstd + (-mean*rstd) - only useful in the
        epilogue when the exps are finished and ACT is idle."""
        t = ts[c]
        if "out_sb" not in t:
            t["out_sb"] = pc.tile([P, NCHUNK, D], F32, name=f"osb{c}",
                                  tag="osb")
        if on_act:
            if "nmb" not in t:
                t["nmb"] = pc.tile([P, NCHUNK], F32, name=f"nmb{c}",
                                   tag="nmb")
            nc.vector.scalar_tensor_tensor(
                out=t["nmb"][:, lo:hi],
                in0=t["mv"][:, lo:hi, 0:1].rearrange("p i o -> p (i o)"),
                scalar=-1.0, in1=t["y"][:, lo:hi], op0=A.mult, op1=A.mult)
            for i in range(lo, hi):
                nc.scalar.activation(
                    out=t["out_sb"][:, i, :], in_=t["t_all"][:, i, :],
                    func=mybir.ActivationFunctionType.Identity,
                    scale=t["y"][:, i:i + 1], bias=t["nmb"][:, i:i + 1])
            return
        for i in range(lo, hi):
            nc.vector.tensor_scalar(
                out=t["out_sb"][:, i, :], in0=t["t_all"][:, i, :],
                scalar1=t["mv"][:, i, 0:1], scalar2=t["y"][:, i:i + 1],
                op0=A.subtract, op1=A.mult)
        if apply_affine:
            for i in range(lo, hi):
                nc.vector.tensor_mul(out=t["out_sb"][:, i, :],
                                     in0=t["out_sb"][:, i, :], in1=gam_tile)
                nc.vector.tensor_add(out=t["out_sb"][:, i, :],
                                     in0=t["out_sb"][:, i, :], in1=bet_tile)

    def store(c, half=None):
        t = ts[c]
        out_r = out_d[c].rearrange("(i p) d -> p i d", p=P)
        if half is None:
            nc.sync.dma_start(out=out_r, in_=t["out_sb"])
        elif half == 0:
            nc.sync.dma_start(out=out_r[:, 0:4, :], in_=t["out_sb"][:, 0:4, :])
        else:
            nc.sync.dma_start(out=out_r[:, 4:8, :], in_=t["out_sb"][:, 4:8, :])

    # ------------- channel-0 minimal critical path to first exp -------------
    alloc_chunk_state(0)
    tr_g(0, "q", 0)
    tr_g(0, "q", 1)
    proj_qk(0, "q", 0)
    proj_qk(0, "q", 1)
    tr_g(0, "k", 0)
    proj_qk(0, "k", 0)
    wpad()
    scores_exp(0, 0)

    # per-slot fillers; emitted AFTER the slot's scores+ctx matmuls.
    # channel 0 finishes its own k/v first; phase-1 for channel c+1 follows
    # the (measured-best) big-piece placement: transposes at i=2/3/4,
    # projections at 5/6, V at 7; the previous channel's tail is split
    # across i=1..5.
    def fillers(c, i):
        if c == 0:
            if i == 0:
                tr_g(0, "k", 1)
                proj_qk(0, "k", 1)
                tr_g(0, "v", 0)
                v_g(0, 0)
            elif i == 1:
                tr_g(0, "v", 1)
                v_g(0, 1)
                loads(1)
            elif i == 2:
                tr_g(1, "q", 0)
            elif i == 3:
                tr_g(1, "q", 1)
                tr_g(1, "k", 0)
            elif i == 4:
                tr_g(1, "k", 1)
                tr_g(1, "v", 0)
            elif i == 5:
                tr_g(1, "v", 1)
                proj_qk(1, "q", 0)
            elif i == 6:
                proj_qk(1, "q", 1)
                proj_qk(1, "k", 0)
            elif i == 7:
                proj_qk(1, "k", 1)
        else:
            # lag-2 ctx pipeline: ctx(c-1, 7) lands in slot (c, 1), so the
            # previous channel's tail shifts one slot later than before
            if i == 0:
                v_g(c, 0)
                v_g(c, 1)
                if c + 1 < C:
                    loads(c + 1)
            elif i == 1:
                tail_a(c - 1)
            elif i == 2:
                fc_group(c - 1, 0)
                if c + 1 < C:
                    tr_g(c + 1, "q", 0)
            elif i == 3:
                fc_group(c - 1, 1)
                if c + 1 < C:
                    tr_g(c + 1, "q", 1)
                    tr_g(c + 1, "k", 0)
            elif i == 4:
                ln_stats(c - 1)
                if c + 1 < C:
                    tr_g(c + 1, "k", 1)
                    tr_g(c + 1, "v", 0)
            elif i == 5:
                ln_out(c - 1, 0, 4)
                store(c - 1, half=0)
                if c + 1 < C:
                    tr_g(c + 1, "v", 1)
                    proj_qk(c + 1, "q", 0)
            elif i == 6:
                ln_out(c - 1, 4, NCHUNK)
                store(c - 1, half=1)
                if c + 1 < C:
                    proj_qk(c + 1, "q", 1)
                    proj_qk(c + 1, "k", 0)
            elif i == 7:
                if c + 1 < C:
                    proj_qk(c + 1, "k", 1)

    # ---------------- software-pipelined channel loop ----------------
    # lag-2 chunk pipeline: vsc/norm_recip(c, i-1) runs behind
    # scores/exp(c, i), and ctx(c, i-2) a full slot later so the PE never
    # waits on the gpsimd normalization round-trip; channel boundaries
    # continue the same cadence (ctx(c-1, 6/7) land in slots (c, 0/1)).
    for c in range(C):
        if c > 0:
            alloc_chunk_state(c)
        st[c]["ctx_ps"] = ctx_psum.tile([D, N], F32, name=f"ctx{c}", tag="ctx")
        for i in range(NCHUNK):
            if not (c == 0 and i == 0):
                scores_exp(c, i)
            if i >= 1:
                vsc_mm(c, i - 1)
            elif c >= 1:
                vsc_mm(c - 1, NCHUNK - 1)
            if i >= 2:
                ctx_mm(c, i - 2)
            elif c >= 1 and i == 0:
                ctx_mm(c - 1, NCHUNK - 2)
            elif c >= 1 and i == 1:
                ctx_mm(c - 1, NCHUNK - 1)
            fillers(c, i)
            flush_reduce()
            wpad()

    # ---------------- pipelined epilogue for channel 3 ----------------
    # per-head norm_recip -> ctx for the last chunk: head h's context
    # matmuls run while exp(h+1..) are still on the ACT engine, instead of
    # the whole chain waiting for the chunk-wide normalization
    c = C - 1
    s, li = st[c], NCHUNK - 1
    ctx_mm(c, NCHUNK - 2)
    for h in range(H):
        j = li * H + h
        hs = slice(DV * h, DV * (h + 1))
        nc.gpsimd.normalize_recip(
            out_ap=s["vsc_all"][:, j, :], in_ap=s["v_nat"][:, li, hs],
            denom_ap=s["ssum"][:, j:j + 1])
        for g in range(2):
            nc.tensor.matmul(
                s["ctx_ps"][hs, g * QT:(g + 1) * QT],
                lhsT=s["vsc_all"][:, j, :],
                rhs=s["e_all"][:, j, g * QT:(g + 1) * QT],
                start=False, stop=True, skip_group_check=True)
    tail_a(c, half=0)
    fc_group(c, 0)
    tail_a(c, half=1)
    ln_stats(c, 0, 4)
    fc_group(c, 1)
    wpad()
    ln_out(c, 0, 4, on_act=True)
    store(c, half=0)
    ln_stats(c, 4, NCHUNK)
    ln_out(c, 4, NCHUNK, on_act=True)
    store(c, half=1)


def _build(apply_affine):
    nc = bacc.Bacc("TRN2", target_bir_lowering=False, debug=False, num_devices=B)
    with tile.TileContext(nc) as tc, ExitStack() as ctx:
        _emit(nc, tc, ctx, apply_affine)
    nc.compile()
    return nc


def kernel(input_Q, input_K, input_V, W_Q, W_K, W_V, W_fc, ln_gamma, ln_beta):
    input_Q = np.ascontiguousarray(np.asarray(input_Q, dtype=np.float32))
    input_K = np.ascontiguousarray(np.asarray(input_K, dtype=np.float32))
    input_V = np.ascontiguousarray(np.asarray(input_V, dtype=np.float32))
    W_Q = np.ascontiguousarray(np.asarray(W_Q, dtype=np.float32))
    W_K = np.ascontiguousarray(np.asarray(W_K, dtype=np.float32))
    W_V = np.ascontiguousarray(np.asarray(W_V, dtype=np.float32))
    W_fc = np.ascontiguousarray(np.asarray(W_fc, dtype=np.float32))
    ln_gamma = np.ascontiguousarray(np.asarray(ln_gamma, dtype=np.float32))
    ln_beta = np.ascontiguousarray(np.asarray(ln_beta, dtype=np.float32))

    apply_affine = not (np.all(ln_gamma == 1.0) and np.all(ln_beta == 0.0))

    key = ("nc", apply_affine)
    if key not in _CACHE:
        _CACHE[key] = _build(apply_affine)
    nc = _CACHE[key]

    wall = np.zeros((P, P + 4 * D + 2), dtype=np.float32)
    wall[:, :P] = np.eye(P, dtype=np.float32)
    for k, W in enumerate((W_Q, W_K, W_V, W_fc)):
        wall[0:D, P + k * D:P + (k + 1) * D] = W.T
    wall[0:D, P + 4 * D] = ln_gamma
    wall[0:D, P + 4 * D + 1] = ln_beta

    in_maps = [
        {"xq": input_Q[b], "xk": input_K[b], "xv": input_V[b], "wall": wall}
        for b in range(B)
    ]
    res = run_bass_kernel_spmd(nc, in_maps, core_ids=list(range(B)))
    return np.stack([res.results[b]["out"] for b in range(B)], axis=0)

